# revision 33
# baseline (speedup 1.0000x reference)
"""Trainium2 Bass kernel for AttentionPatcher (GQA attention block, S=2048).

Sharding: 8-way tensor parallel over KV head groups. Core c owns KV head c
and query heads 4c..4c+3: it computes its Q/K/V projections, RoPE, causal
attention, and a full partial o_proj (wo column shard); a per-s-tile
ReduceScatter(add) over the 8 cores then leaves core c with rows
[512c, 512c+512) of the final output, which the host concatenates.

v11 on top of v3 (bf16 matmuls, SBUF-resident weights, interleaved phases,
2-bank PSUM pairs, approx reciprocal):
 - softmax denominator from vector-engine block pre-sums instead of one
   ones-matmul per l-block: the two l-blocks of a pair are added on DVE
   (bf16), off-diagonal pair-sums are further added pairwise into f32r
   quad-sums, and a single ones-matmul per group hits PSUM (f32r runs at
   bf16 speed for >=256 columns; rounding averages out in positive sums)
 - attn and o_proj instruction streams interleaved pair-by-pair (PE is
   in-order; phase-granular interleaving left it idle on exp chains);
   rope(3) drains against oproj(0)'s held-back tail units
 - qkv(3) split into two passes over a re-streamed x: the first carries
   only q0/q1 (one psum pair) so attn(0) interleaves into its matmul
   stream at depth 2 -- the attn(0) exp chains vanish under qkv matmuls
 - phase-boundary psum evictions split across scalar (Identity with AP
   bias, bq pre-scaled by 1/sqrt(D)) and vector, halving the serial
   latency before the next phase's psum slots free
 - diagonal-pair causal mask applied as ONE DVE op over both halves
 - first QKV chunk loaded ko-at-a-time (first matmul gates on ~300KB;
   finer backlogs the sequencer's descriptor launches); o_proj outputs
   written as mo-pairs (2KB/partition)

Measured: 393.9us HW best-of-3 (baseline v3: 418.4us); rel err 6.76e-3.
PE busy ~366us of ~350us theoretical minimum at bf16 (1 col/cycle
@2.4GHz); mid-kernel PE idle ~12us, startup ~10us, tail ~6us.
Rejected by measurement: fp8e4 DoubleRow denominator (gpsimd fp8 cast is
software-emulated, ~16us per [128,2,512] tile, and produced NaNs via the
den path), additive pre-exp mask on PSUM (DVE psum RMW runs at f32 base
rate, longer than two post-exp bf16 SBUF multiplies), diagonal mask half
on gpsimd (Pool elementwise ~1.1us on the chain regressed HW ~13us),
evicting k/v before the q heads at phase boundaries (+70us!! -- the next
phase's first matmuls gate in-order on the q psum slots, and the rope
drain queue reorders behind them).
"""
import os
import sys

import numpy as np

if os.path.isdir("/opt/trn_rl_repo") and "/opt/trn_rl_repo" not in sys.path:
    sys.path.insert(0, "/opt/trn_rl_repo")

import ml_dtypes

import concourse.bacc as bacc
import concourse.mybir as mybir
import concourse.tile as tile
from concourse.bass_utils import run_bass_kernel_spmd
from concourse.masks import make_identity

F32 = mybir.dt.float32
F32R = mybir.dt.float32r
BF16 = mybir.dt.bfloat16
ActF = mybir.ActivationFunctionType
Alu = mybir.AluOpType
NPBF = ml_dtypes.bfloat16

H, KV, D, S = 32, 8, 128, 2048
HID = H * D
NCORES = 8
G = H // KV          # query heads per core
ST = 512             # s-tile size
NST = S // ST        # 4 s-tiles
KO = HID // 128      # 32 contraction subtiles
MO = HID // 128      # 32 output row tiles
MP = MO // 2         # 16 output row-pair tiles
XC = 4               # x-tile DMA chunk (ko per descriptor)
INV_SQRT_D = 1.0 / float(np.sqrt(D))


def build_nc(with_collective=True):
    nc = bacc.Bacc("TRN2", target_bir_lowering=False, debug=False)

    x = nc.dram_tensor("x", [NST, 128, KO, ST], BF16, kind="ExternalInput")
    wq = nc.dram_tensor("wq", [128, KO, G * 128], BF16, kind="ExternalInput")
    wk = nc.dram_tensor("wk", [128, KO, 128], BF16, kind="ExternalInput")
    wv = nc.dram_tensor("wv", [128, KO, 128], BF16, kind="ExternalInput")
    wo = nc.dram_tensor("wo", [128, MO, G, 128], BF16, kind="ExternalInput")
    bq = nc.dram_tensor("bq", [128, G], F32, kind="ExternalInput")
    bk = nc.dram_tensor("bk", [128, 1], F32, kind="ExternalInput")
    bv = nc.dram_tensor("bv", [128, 1], F32, kind="ExternalInput")
    cos = nc.dram_tensor("cos", [128, S], F32, kind="ExternalInput")
    sin = nc.dram_tensor("sin", [128, S], F32, kind="ExternalInput")
    rot = nc.dram_tensor("rot", [128, 128], F32R, kind="ExternalInput")
    # row-pair layout: yout[j, p, h, s] = y row (2j+h)*128+p of this core's
    # ReduceScatter shard (host maps to global row (4c+2j+h)*128+p)
    yout = nc.dram_tensor("y", [2, 128, 2, S], BF16, kind="ExternalOutput")

    with tile.TileContext(nc) as tc:
        with (
            tc.tile_pool(name="const", bufs=1) as const,
            tc.tile_pool(name="sb", bufs=3) as sb,
            tc.tile_pool(name="ps", bufs=1, space="PSUM") as ps,
            tc.tile_pool(name="dram", bufs=1, space="DRAM") as dram,
        ):
            # ---- resident weights/tables ----
            wq_sb = const.tile([128, KO, G * 128], BF16)   # 32K/part
            wk_sb = const.tile([128, KO, 128], BF16)       # 8K
            wv_sb = const.tile([128, KO, 128], BF16)       # 8K
            wo_sb = const.tile([128, MO, G, 128], BF16)    # 32K
            # wk/wv/wq chunks are loaded just-in-time inside emit_qkv(0);
            # cos/sin/rot/bias loads are emitted AFTER emit_qkv(0): QKV(0)
            # already needs wq(8M)+x(4M)+wk/wv(2M) and sits right at the HBM
            # bandwidth budget; these are first used at the qkv(0) boundary
            cos_sb = const.tile([128, S], F32)
            sin_sb = const.tile([128, S], F32)
            rot_sb = const.tile([128, 128], F32R)
            bq_sb = const.tile([128, G], F32)
            bk_sb = const.tile([128, 1], F32)
            bv_sb = const.tile([128, 1], F32)

            def emit_tables_load():
                # small tensors first: the qkv(0) boundary evictions read the
                # biases, and the first rope matmul reads rot
                nc.scalar.dma_start(rot_sb[:], rot[:, :])
                nc.scalar.dma_start(bq_sb[:], bq[:, :])
                nc.scalar.dma_start(bk_sb[:], bk[:, :])
                nc.scalar.dma_start(bv_sb[:], bv[:, :])
                nc.scalar.dma_start(cos_sb[:], cos[:, :])
                nc.scalar.dma_start(sin_sb[:], sin[:, :])

            ones_bf = const.tile([128, 128], BF16)
            nc.vector.memset(ones_bf[:], 1.0)
            ones_r = const.tile([128, 128], F32R)
            nc.vector.tensor_copy(ones_r[:], ones_bf[:])
            ident_f = const.tile([128, 128], F32)
            make_identity(nc, ident_f)
            ident_bf = const.tile([128, 128], BF16)
            nc.vector.tensor_copy(ident_bf[:], ident_f[:])
            # static causal masks for the 4 diagonal l-blocks of an s-tile:
            # mask_j[l, s] = 1 where s - l >= 128j else 0
            cmask = const.tile([128, 4, ST], BF16)
            nc.vector.memset(cmask[:], 1.0)
            for j in range(4):
                nc.gpsimd.affine_select(
                    out=cmask[:, j, :], in_=cmask[:, j, :],
                    compare_op=Alu.is_ge, fill=0.0,
                    base=-128 * j, channel_multiplier=-1,
                    pattern=[[1, ST]],
                )

            # ---- resident activations (all bf16) ----
            k_rot = const.tile([128, S], BF16)             # K, (d, l)
            q_rot = const.tile([128, G, S], BF16)          # Q, (d, g, s)
            v_t = const.tile([128, S // 128, 128], BF16)   # V^T
            out_t = const.tile([128, G, S], BF16)          # attn out

            cc_in = dram.tile([NST, MP, 128, 2, ST], BF16)
            cc_out = dram.tile([NST, 2, 128, 2, ST], BF16)

            # PSUM discipline: every psum tile is a [128, 2, ST] f32 "pair"
            # (2 banks); the 4 pair bufs cover all 8 banks. Halves act as
            # independent accumulators. Live set never exceeds 4:
            #   qkv: q01 + q23 + kv (+1 free slot for rope/vt drains)
            #   attn standalone: 3 score pairs in flight + 1 avden acc
            #   attn x oproj interleave: 2 score + 1 acc + 1 y2
            def pair(name):
                return ps.tile([128, 2, ST], F32, tag="pair", bufs=4,
                               name=name)

            # deferred PE work (rope matmuls / V transposes) injected into
            # a later qkv matmul stream so the PE queue never stalls on the
            # vector-engine eviction chain at a phase boundary
            pending = []

            def drain(n=1):
                for _ in range(n):
                    if pending:
                        pending.pop(0)()

            def rope_cb(raw, dst_ap, sl):
                def cb():
                    ps_r = pair("ps_rot")
                    nc.tensor.matmul(ps_r[:, 0, :], rot_sb[:], raw[:],
                                     start=True, stop=True)
                    t1 = sb.tile([128, ST], F32, tag="rt1", bufs=2)
                    t2 = sb.tile([128, ST], F32, tag="rt2", bufs=2)
                    # split across engines: gpsimd takes the SBUF-only ops
                    nc.gpsimd.tensor_tensor(t1[:], raw[:], cos_sb[:, sl],
                                            Alu.mult)
                    nc.vector.tensor_tensor(t2[:], ps_r[:, 0, :],
                                            sin_sb[:, sl], Alu.mult)
                    nc.gpsimd.tensor_tensor(dst_ap, t1[:], t2[:], Alu.add)
                return cb

            def vt_cb(v_sb, si):
                def cb():
                    ps_t = ps.tile([128, 4, 128], BF16, tag="pair", bufs=4,
                                   name="ps_t")
                    for j in range(4):
                        nc.tensor.transpose(ps_t[:, j, :],
                                            v_sb[:, j * 128:(j + 1) * 128],
                                            ident_bf[:])
                    nc.vector.tensor_copy(v_t[:, si * 4:(si + 1) * 4, :],
                                          ps_t[:])
                return cb

            def gen_qkv(si, gs=(0, 1, 2, 3), do_kv=True, do_drain=True):
                # one pass over the x(si) stream accumulating q heads `gs`
                # (and k/v if do_kv). qkv(3) runs as two passes so its first
                # half needs only one q psum pair, freeing banks to
                # interleave attn(0) into its matmul stream.
                sl = slice(si * ST, (si + 1) * ST)
                ps_q = {}
                if 0 in gs:
                    ps_q01 = pair("ps_q01")
                    ps_q[0], ps_q[1] = ps_q01[:, 0, :], ps_q01[:, 1, :]
                if 2 in gs:
                    ps_q23 = pair("ps_q23")
                    ps_q[2], ps_q[3] = ps_q23[:, 0, :], ps_q23[:, 1, :]
                ps_kv = pair("ps_kv") if do_kv else None
                for kc in range(KO // XC):
                    xt = sb.tile([128, XC, ST], BF16, tag="x", bufs=3)
                    if si == 0 and kc == 0:
                        # ko-granular first chunk: the first matmul only
                        # gates on ~300KB instead of ~1.5MB (finer than this
                        # backlogs the sequencer's descriptor launches)
                        for u in range(XC):
                            ko = kc * XC + u
                            nc.sync.dma_start(xt[:, u:u + 1, :],
                                              x[si][:, ko:ko + 1])
                            nc.scalar.dma_start(wq_sb[:, ko:ko + 1, :],
                                                wq[:, ko:ko + 1, :])
                            if u == 0 and kc == 0:
                                nc.scalar.dma_start(wk_sb[:, 0:1, :],
                                                    wk[:, 0:1, :])
                                nc.scalar.dma_start(wv_sb[:, 0:1, :],
                                                    wv[:, 0:1, :])
                            if u == 1 and kc == 0:
                                nc.scalar.dma_start(wk_sb[:, 1:8, :],
                                                    wk[:, 1:8, :])
                                nc.scalar.dma_start(wv_sb[:, 1:8, :],
                                                    wv[:, 1:8, :])
                    else:
                        if si == 0:
                            wsl = slice(kc * XC, (kc + 1) * XC)
                            nc.scalar.dma_start(wq_sb[:, wsl, :],
                                                wq[:, wsl, :])
                            if kc % 2 == 0:
                                c4 = kc // 2
                                ksl = slice(c4 * (KO // 4),
                                            (c4 + 1) * (KO // 4))
                                nc.scalar.dma_start(wk_sb[:, ksl, :],
                                                    wk[:, ksl, :])
                                nc.scalar.dma_start(wv_sb[:, ksl, :],
                                                    wv[:, ksl, :])
                        if si == 1 and kc % 2 == 1:
                            # o_proj weights ride the qkv(1) window (x-only
                            # there, so DMA headroom; first consumed by
                            # oproj(0), several phases later)
                            c4 = kc // 2
                            msl = slice(c4 * (MO // 4), (c4 + 1) * (MO // 4))
                            nc.scalar.dma_start(wo_sb[:, msl], wo[:, msl])
                        nc.sync.dma_start(xt[:],
                                          x[si][:, kc * XC:(kc + 1) * XC])
                    for u in range(XC):
                        ko = kc * XC + u
                        st = (ko == 0)
                        sp = (ko == KO - 1)
                        for g in gs:
                            nc.tensor.matmul(
                                ps_q[g],
                                wq_sb[:, ko, g * 128:(g + 1) * 128],
                                xt[:, u, :], start=st, stop=sp)
                        if do_kv:
                            nc.tensor.matmul(ps_kv[:, 0, :],
                                             wk_sb[:, ko, :],
                                             xt[:, u, :], start=st, stop=sp)
                            nc.tensor.matmul(ps_kv[:, 1, :],
                                             wv_sb[:, ko, :],
                                             xt[:, u, :], start=st, stop=sp)
                        if do_drain:
                            drain()
                    yield
                if si == 0:
                    emit_tables_load()
                # boundary: evict psums on vector, defer the PE-side rope /
                # transpose work into a later qkv matmul stream
                # evictions split across scalar/vector so psum slots free
                # in ~half the serial-vector time (the next phase's first
                # matmuls gate on these; bq comes pre-scaled by 1/sqrt(D))
                for g in gs:
                    q_raw = sb.tile([128, ST], F32R, tag="q_raw", bufs=6,
                                    name=f"q_raw{g}")
                    if g % 2 == 0:
                        nc.scalar.activation(q_raw[:], ps_q[g], ActF.Identity,
                                             bias=bq_sb[:, g:g + 1],
                                             scale=INV_SQRT_D)
                    else:
                        nc.vector.tensor_scalar(q_raw[:], ps_q[g],
                                                INV_SQRT_D,
                                                bq_sb[:, g:g + 1],
                                                Alu.mult, Alu.add)
                    pending.append(rope_cb(q_raw, q_rot[:, g, sl], sl))
                if do_kv:
                    k_raw = sb.tile([128, ST], F32R, tag="k_raw", bufs=2)
                    nc.vector.tensor_scalar(k_raw[:], ps_kv[:, 0, :],
                                            bk_sb[:, 0:1], None, Alu.add)
                    pending.append(rope_cb(k_raw, k_rot[:, sl], sl))
                    v_sb = sb.tile([128, ST], BF16, tag="v_sb", bufs=2)
                    nc.scalar.activation(v_sb[:], ps_kv[:, 1, :],
                                         ActF.Identity, bias=bv_sb[:, 0:1])
                    pending.append(vt_cb(v_sb, si))

            def emit_qkv(si):
                run(gen_qkv(si))

            def gen_attn(si, depth):
                # one softmax pipeline across all G heads of this s-tile,
                # yielding after each scores pair so o_proj matmuls can be
                # interleaved into the PE stream; `depth` pairs stay in
                # flight so the exp/mask/sum chain hides under matmuls
                nli = (si + 1) * (ST // 128)
                sl = slice(si * ST, (si + 1) * ST)
                hist = []
                acc = {}   # g -> avden pair

                def avden(rec):
                    g, p2, den_mm, offs = rec
                    if g not in acc:
                        acc[g] = pair(f"ps_avden{g}")
                    ps_av, ps_den = acc[g][:, 0, :], acc[g][:, 1, :]
                    for h in range(2):
                        li, off = offs[h]
                        nc.tensor.matmul(ps_av[:, off:], v_t[:, li, :],
                                         p2[:, h, off:],
                                         start=(li == 0),
                                         stop=(li == nli - 1))
                    if den_mm is not None:
                        kind, src, off0, st, sp = den_mm
                        ones = ones_bf if kind == "bf" else ones_r
                        nc.tensor.matmul(ps_den[:, off0:], ones[:],
                                         src[:, off0:], start=st, stop=sp)
                    if offs[1][0] == nli - 1:
                        # head done: normalize and release its accumulators
                        recip = sb.tile([128, ST], F32, tag="recip", bufs=2)
                        nc.vector.reciprocal_approx_fast(recip[:], ps_den[:])
                        nc.vector.tensor_tensor(out_t[:, g, sl], ps_av[:],
                                                recip[:], Alu.mult)
                        del acc[g]

                for g in range(G):
                    prev_wsum = None
                    npairs = nli // 2
                    for pi in range(npairs):
                        ps_s2 = pair("ps_s2")
                        # both halves of a pair share the h=0 column base so
                        # one DVE add covers the pair for the denominator;
                        # the h=1 diagonal block's extra columns are real
                        # scores that cmask zeroes (no junk reaches the sum)
                        j0 = 2 * pi - si * (ST // 128)
                        off0 = 128 * j0 if j0 > 0 else 0
                        offs = []
                        for h in range(2):
                            li = 2 * pi + h
                            j = li - si * (ST // 128)
                            off = 128 * j if j > 0 else 0
                            offs.append((li, off))
                            nc.tensor.matmul(
                                ps_s2[:, h, off0:],
                                k_rot[:, li * 128:(li + 1) * 128],
                                q_rot[:, g, si * ST + off0:(si + 1) * ST],
                                start=True, stop=True)
                        p2 = sb.tile([128, 2, ST], BF16, tag="p", bufs=4)
                        nc.scalar.activation(p2[:, :, off0:],
                                             ps_s2[:, :, off0:], ActF.Exp)
                        diag = j0 >= 0
                        if diag:
                            # causal within the diagonal s-tile: zero where
                            # s - l < 128j; cmask[j0:j0+2] is exactly the
                            # pair's mask, so one DVE op covers both halves
                            nc.vector.tensor_tensor(
                                p2[:, :, off0:], p2[:, :, off0:],
                                cmask[:, j0:j0 + 2, off0:], Alu.mult)
                        # denominator pre-sum: one bf16 l-block pair sum;
                        # off-diagonal pairs further collapse into an f32r
                        # quad-sum so one matmul covers four l-blocks
                        wsum = sb.tile([128, ST], BF16, tag="wsum", bufs=3)
                        nc.vector.tensor_tensor(wsum[:, off0:],
                                                p2[:, 0, off0:],
                                                p2[:, 1, off0:], Alu.add)
                        if diag:
                            den_mm = ("bf", wsum, off0,
                                      pi == 0, pi == npairs - 1)
                        elif pi % 2 == 1:
                            qsum = sb.tile([128, ST], F32R, tag="qsum",
                                           bufs=2)
                            nc.vector.tensor_tensor(qsum[:], prev_wsum[:],
                                                    wsum[:], Alu.add)
                            den_mm = ("r", qsum, 0, pi == 1, False)
                        else:
                            den_mm = None
                        prev_wsum = wsum
                        hist.append((g, p2, den_mm, offs))
                        if len(hist) >= depth:
                            avden(hist.pop(0))
                        yield
                while hist:
                    avden(hist.pop(0))

            def gen_oproj(si):
                sl = slice(si * ST, (si + 1) * ST)
                for mp in range(MP):
                    ps_y2 = pair("ps_y2")
                    y2 = sb.tile([128, 2, ST], BF16, tag="y2", bufs=4)
                    for h in range(2):
                        mo = 2 * mp + h
                        for g in range(G):
                            nc.tensor.matmul(ps_y2[:, h, :],
                                             wo_sb[:, mo, g, :],
                                             out_t[:, g, sl],
                                             start=(g == 0),
                                             stop=(g == G - 1))
                        if h == 0:
                            nc.scalar.activation(y2[:, 0, :], ps_y2[:, 0, :],
                                                 ActF.Copy)
                        else:
                            nc.vector.tensor_copy(y2[:, 1, :], ps_y2[:, 1, :])
                    # one mo-pair write: 2KB per partition line
                    if not with_collective and mp < 2:
                        # profiling build: these row-blocks are the local
                        # stand-in for the ReduceScatter output
                        nc.sync.dma_start(yout[mp][:, :, sl], y2[:])
                    else:
                        qeng = (nc.sync, nc.scalar, nc.gpsimd)[mp % 3]
                        qeng.dma_start(cc_in[si, mp], y2[:])
                    yield
                if with_collective:
                    # core c receives row-pairs mp = 2c..2c+1 of this s-tile
                    nc.gpsimd.collective_compute(
                        "ReduceScatter",
                        Alu.add,
                        replica_groups=[list(range(NCORES))],
                        ins=[cc_in[si].opt()],
                        outs=[cc_out[si].opt()],
                    )
                    nc.sync.dma_start(yout[:, :, :, sl], cc_out[si])

            def run(gen):
                for _ in gen:
                    pass

            def take(gen, n):
                for _ in range(n):
                    try:
                        yield next(gen)
                    except StopIteration:
                        return

            def gen_drain():
                while pending:
                    pending.pop(0)()
                    yield

            def chain(*gens):
                for g in gens:
                    yield from g

            def interleave(ga, na, gb, nb):
                # alternate na steps of ga with nb steps of gb until both
                # are exhausted (PE is in-order: this IS the schedule)
                da = db = False
                while not (da and db):
                    for _ in range(na):
                        try:
                            next(ga)
                        except StopIteration:
                            da = True
                            break
                    for _ in range(nb):
                        try:
                            next(gb)
                        except StopIteration:
                            db = True
                            break

            # rope(i) callbacks (pushed at qkv(i)'s boundary) drain into the
            # next qkv phase's matmul stream, where the 4th PSUM pair slot is
            # free; rope(3) drains standalone before attn(2) -- its results
            # are only needed by attn(3), so nothing waits on the chain
            emit_qkv(0)
            emit_qkv(1)
            # qkv(2) and qkv(3) each run as two passes over a re-streamed x:
            # the first carries only q0/q1 (one psum pair), so attention of
            # an earlier tile interleaves into its dense matmul stream
            # (2 score pairs + acc + q01 = 4 pair slots); the second pass
            # carries q2/q3 + k/v and drains the deferred rope work
            interleave(gen_qkv(2, (0, 1), False, False), 1, gen_attn(0, 2), 1)
            run(gen_qkv(2, (2, 3), True, True))
            interleave(gen_qkv(3, (0, 1), False, False), 1, gen_attn(1, 2), 2)
            run(gen_qkv(3, (2, 3), True, True))
            op0 = gen_oproj(0)
            interleave(gen_attn(2, 2), 2, take(op0, 12), 1)
            # rope(3) drains against oproj(0)'s held-back tail units so the
            # PE has matmul work while the rope chains run
            interleave(gen_drain(), 1, op0, 1)
            interleave(gen_attn(3, 2), 1, chain(gen_oproj(1), gen_oproj(2)), 1)
            run(gen_oproj(3))

    nc.compile()
    return nc


def _rot_matrix():
    # q_rot = R @ q with rotate_half along D: R @ v = concat(-v[64:], v[:64])
    R = np.zeros((128, 128), np.float32)
    for i in range(64):
        R[i, 64 + i] = -1.0
        R[64 + i, i] = 1.0
    return R


def _prep_in_maps(inputs):
    x = np.ascontiguousarray(np.asarray(inputs["hidden_states"],
                                        np.float32)[0, :, 0, :])
    wq = np.asarray(inputs["wq"], np.float32)
    wk = np.asarray(inputs["wk"], np.float32)
    wv = np.asarray(inputs["wv"], np.float32)
    wo = np.asarray(inputs["wo"], np.float32)
    bq = np.asarray(inputs["bq"], np.float32)
    bk = np.asarray(inputs["bk"], np.float32)
    bv = np.asarray(inputs["bv"], np.float32)
    cos_t = np.ascontiguousarray(np.asarray(inputs["cos_t"],
                                            np.float32)[0, 0])  # (128, S)
    sin_t = np.ascontiguousarray(np.asarray(inputs["sin_t"], np.float32)[0, 0])
    rotT = np.ascontiguousarray(_rot_matrix().T)

    # x tiled as [si, p, ko, ST], contiguous per (si, ko-chunk)
    x_t = np.ascontiguousarray(
        x.reshape(KO, 128, NST, ST).transpose(2, 1, 0, 3).astype(NPBF))
    in_maps = []
    for c in range(NCORES):
        qs = slice(c * G * 128, (c + 1) * G * 128)
        ks = slice(c * 128, (c + 1) * 128)
        wq_t = np.ascontiguousarray(
            wq[qs].T.reshape(KO, 128, G * 128).transpose(1, 0, 2)
            .astype(NPBF))
        wk_t = np.ascontiguousarray(
            wk[ks].T.reshape(KO, 128, 128).transpose(1, 0, 2).astype(NPBF))
        wv_t = np.ascontiguousarray(
            wv[ks].T.reshape(KO, 128, 128).transpose(1, 0, 2).astype(NPBF))
        # wo column shard -> (d, mo, g, m): woT[g*128+d, mo*128+m]
        wo_t = np.ascontiguousarray(
            wo[:, qs].T.reshape(G, 128, MO, 128).transpose(1, 2, 0, 3)
            .astype(NPBF))
        in_maps.append({
            "x": x_t,
            "wq": wq_t,
            "wk": wk_t,
            "wv": wv_t,
            "wo": wo_t,
            "bq": np.ascontiguousarray(bq[qs].reshape(G, 128).T
                                       * np.float32(INV_SQRT_D)),
            "bk": np.ascontiguousarray(bk[ks][:, None]),
            "bv": np.ascontiguousarray(bv[ks][:, None]),
            "cos": cos_t,
            "sin": sin_t,
            "rot": rotT,
        })
    return in_maps


_NC = None


def _get_nc():
    global _NC
    if _NC is None:
        _NC = build_nc()
    return _NC


def assemble_output(results):
    """Per-s-tile ReduceScatter: core c holds y row-pairs mp = 2c..2c+1,
    i.e. rows (4c+2j+h)*128+p at yc[j, p, h]."""
    y = np.empty((HID, S), np.float32)
    for c in range(NCORES):
        yc = np.asarray(results[c]["y"], dtype=np.float32)  # [2, 128, 2, S]
        for j in range(yc.shape[0]):
            for h in range(2):
                mo = 4 * c + 2 * j + h
                y[mo * 128:(mo + 1) * 128] = yc[j, :, h]
    return y[None, :, None, :]


def kernel(**inputs):
    nc = _get_nc()
    in_maps = _prep_in_maps(inputs)
    res = run_bass_kernel_spmd(nc, in_maps, core_ids=list(range(NCORES)))
    return assemble_output(res.results)
